# revision 9
# baseline (speedup 1.0000x reference)
"""Performer attention TRN2 kernel.

Math per (b,h):
  k_feats[s,r] = exp(k[s]@W[:,r] - 0.5*||k[s]||^2)     (1/sqrt(R) dropped: cancels)
  q_feats[n,r] = exp(q[n]@W[:,r] - 0.5*||q[n]||^2)
  kv[r,d]  = sum_s k_feats[s,r] * v[s,d];  ksum[r] = sum_s k_feats[s,r]
  out[n,d] = (q_feats[n,:]@kv[:,d]) / (q_feats[n,:]@ksum)

Sharding: 16 (b,h) pairs, 2 per core across 8 cores.

Device layouts (per core, m in {0,1} is the local bh index):
  q_aug [2, 66, T]  f32: rows 0..63 = q^T (e on partitions), rows 64/65 =
                         hi/lo split of -0.5*||q||^2 (applied via the
                         contraction against ones-rows of w_aug)
  k_t   [2, 64, T]  f32: k^T
  k_nrm [2, 128, 64] f32: -0.5*||k||^2 arranged [partition=t%128, tile=t//128]
  v_r   [2, 128, 64*65] bf16: v with ones column, tiled [t%128, t//128, 65]
  w_aug [2, 66, R]  f32: W with two ones-rows appended
  out   [2, 128, 64*64] f32: out tiled [t%128, t//128, 64]
"""

import sys

for _p in ("/opt/trn_rl_repo",):
    if _p not in sys.path:
        sys.path.insert(0, _p)

import numpy as np
import ml_dtypes

B, T, H, E = 2, 8192, 8, 64
R = 256
NBH = B * H  # 16
N_CORES = 8
MPC = NBH // N_CORES  # bh pairs per core = 2
ST = 128  # rows per s/n tile
N_ST = T // ST  # 64
NCHUNK = 512  # q-side moving chunk
N_CH = T // NCHUNK  # 16
KAUG = E + 2  # 66

_NC_CACHE = {}


def _build_nc():
    import concourse.bass as bass
    import concourse.mybir as mybir
    import concourse.tile as tile

    f32 = mybir.dt.float32
    f32r = mybir.dt.float32r
    bf16 = mybir.dt.bfloat16
    Exp = mybir.ActivationFunctionType.Exp

    nc = bass.Bass(target_bir_lowering=False)

    qa = nc.declare_dram_parameter("q_aug", [MPC, KAUG, T], f32r, isOutput=False)
    kt = nc.declare_dram_parameter("k_t", [MPC, E, T], f32r, isOutput=False)
    kn = nc.declare_dram_parameter("k_nrm", [MPC, ST, N_ST], f32, isOutput=False)
    vr = nc.declare_dram_parameter("v_r", [MPC, ST, N_ST * 65], bf16, isOutput=False)
    wa = nc.declare_dram_parameter("w_aug", [MPC, KAUG, R], f32r, isOutput=False)
    ou = nc.declare_dram_parameter("out", [MPC, ST, N_ST * E], f32, isOutput=True)

    with tile.TileContext(nc) as tc:
        with (
            tc.tile_pool(name="io", bufs=2) as io,
            tc.tile_pool(name="feats", bufs=3) as feats,
            tc.tile_pool(name="small", bufs=2) as small,
            tc.tile_pool(name="ps", bufs=2, space="PSUM") as ps,
            tc.tile_pool(name="acc", bufs=1, space="PSUM") as accp,
        ):
            for m in range(MPC):
                # ---- input DMAs ----
                w_sb = io.tile([KAUG, R], f32r, name=f"w_sb")
                nc.sync.dma_start(out=w_sb[:], in_=wa[m])
                kn_sb = io.tile([ST, N_ST], f32, name=f"kn_sb")
                nc.sync.dma_start(out=kn_sb[:], in_=kn[m])
                v_sb = io.tile([ST, N_ST * 65], bf16, name=f"v_sb")
                for c in range(2):
                    nc.sync.dma_start(
                        out=v_sb[:, c * 2080 : (c + 1) * 2080],
                        in_=vr[m, :, c * 2080 : (c + 1) * 2080],
                    )
                k_sb = io.tile([E, T], f32r, name=f"k_sb", bufs=1)
                for c in range(4):
                    nc.sync.dma_start(
                        out=k_sb[:, c * 2048 : (c + 1) * 2048],
                        in_=kt[m, :, c * 2048 : (c + 1) * 2048],
                    )
                q_sb = io.tile([KAUG, T], f32r, name=f"q_sb")
                for c in range(4):
                    nc.sync.dma_start(
                        out=q_sb[:, c * 2048 : (c + 1) * 2048],
                        in_=qa[m, :, c * 2048 : (c + 1) * 2048],
                    )

                # ---- phase A: k_feats tiles + kv/ksum accumulation ----
                kv_acc = [
                    accp.tile([ST, 65], f32, name=f"kv_acc{h}") for h in range(2)
                ]
                for st in range(N_ST):
                    ps_k = ps.tile([ST, R], f32, name="ps_k")
                    nc.tensor.matmul(
                        ps_k[:],
                        lhsT=k_sb[:, st * ST : (st + 1) * ST],
                        rhs=w_sb[0:E, :],
                        start=True,
                        stop=True,
                    )
                    kf = feats.tile([ST, R], bf16, name="kf")
                    nc.scalar.activation(
                        kf[:], ps_k[:], Exp, bias=kn_sb[:, st : st + 1], scale=1.0
                    )
                    for h in range(2):
                        nc.tensor.matmul(
                            kv_acc[h][:],
                            lhsT=kf[:, h * 128 : (h + 1) * 128],
                            rhs=v_sb[:, st * 65 : (st + 1) * 65],
                            start=(st == 0),
                            stop=(st == N_ST - 1),
                        )
                kv_sb = [small.tile([ST, 65], bf16, name=f"kv_sb{h}") for h in range(2)]
                for h in range(2):
                    nc.vector.tensor_copy(kv_sb[h][:], kv_acc[h][:])

                # ---- phase B: q_feats chunks + out tiles ----
                out_sb = io.tile([ST, N_ST * E], f32, name="out_sb")
                for ck in range(N_CH):
                    qf = []
                    for h in range(2):
                        ps_q = ps.tile([ST, NCHUNK], f32, name="ps_q")
                        nc.tensor.matmul(
                            ps_q[:],
                            lhsT=w_sb[:, h * 128 : (h + 1) * 128],
                            rhs=q_sb[:, ck * NCHUNK : (ck + 1) * NCHUNK],
                            start=True,
                            stop=True,
                        )
                        qf_h = feats.tile([ST, NCHUNK], bf16, name="qf")
                        nc.scalar.activation(qf_h[:], ps_q[:], Exp)
                        qf.append(qf_h)
                    for nt in range(NCHUNK // ST):
                        gt = ck * (NCHUNK // ST) + nt  # global n-tile index
                        ps_o = ps.tile([ST, 65], f32, name="ps_o")
                        for h in range(2):
                            nc.tensor.matmul(
                                ps_o[:],
                                lhsT=qf[h][:, nt * ST : (nt + 1) * ST],
                                rhs=kv_sb[h][:],
                                start=(h == 0),
                                stop=(h == 1),
                            )
                        rec = small.tile([ST, 1], f32, name="rec")
                        nc.vector.reciprocal(rec[:], ps_o[:, 64:65])
                        nc.vector.tensor_scalar_mul(
                            out_sb[:, gt * E : (gt + 1) * E], ps_o[:, 0:E], rec[:]
                        )
                    if (ck + 1) % 4 == 0:
                        g = ck // 4
                        nc.sync.dma_start(
                            out=ou[m, :, g * 1024 : (g + 1) * 1024],
                            in_=out_sb[:, g * 1024 : (g + 1) * 1024],
                        )
    # TRN2 codegen allows at most 1 sem wait per instruction; split extras
    # into InstEventSemaphore (the Bacc legalization pass, safe on Bass).
    from concourse import bacc

    bacc._bass_rust.generate_event_semaphores(nc)
    return nc


def _prep_inputs(query, key, value, weights):
    query = np.asarray(query, dtype=np.float32)
    key = np.asarray(key, dtype=np.float32)
    value = np.asarray(value, dtype=np.float32)
    weights = np.asarray(weights, dtype=np.float32)

    def aug_qk(x):
        # x [B,T,H,E] -> xt [NBH, E, T], nrm [NBH, T]
        xt = np.ascontiguousarray(x.transpose(0, 2, 3, 1)).reshape(NBH, E, T)
        nrm = (-0.5 * np.sum(x.astype(np.float64) * x, axis=3)).astype(np.float32)
        nrm = np.ascontiguousarray(nrm.transpose(0, 2, 1)).reshape(NBH, T)
        return xt, nrm

    qt, qn = aug_qk(query)
    # hi/lo split of the q norm so reduced-precision matmul modes keep it exact
    hi = (qn.view(np.uint32) & np.uint32(0xFFFFE000)).view(np.float32)
    lo = qn - hi
    q_aug = np.concatenate([qt, hi[:, None, :], lo[:, None, :]], axis=1)
    q_aug = np.ascontiguousarray(q_aug)  # [NBH, 66, T]

    kt, knrm = aug_qk(key)
    kt = np.ascontiguousarray(kt)  # [NBH, 64, T]
    k_nrm = np.ascontiguousarray(
        knrm.reshape(NBH, N_ST, ST).transpose(0, 2, 1)
    )  # [NBH, 128, 64]

    vt = value.transpose(0, 2, 1, 3).reshape(NBH, N_ST, ST, E)
    v_aug = np.concatenate([vt, np.ones((NBH, N_ST, ST, 1), np.float32)], axis=3)
    v_r = np.ascontiguousarray(v_aug.transpose(0, 2, 1, 3)).reshape(
        NBH, ST, N_ST * 65
    ).astype(ml_dtypes.bfloat16)

    w_aug = np.concatenate(
        [weights.reshape(NBH, E, R), np.ones((NBH, 2, R), np.float32)], axis=1
    )
    w_aug = np.ascontiguousarray(w_aug)  # [NBH, 66, R]
    return q_aug, kt, k_nrm, v_r, w_aug


def _run(inputs, trace=False):
    from concourse.bass_utils import run_bass_kernel_spmd

    q_aug, kt, k_nrm, v_r, w_aug = _prep_inputs(
        inputs["query"], inputs["key"], inputs["value"], inputs["weights"]
    )
    if "nc" not in _NC_CACHE:
        _NC_CACHE["nc"] = _build_nc()
    nc = _NC_CACHE["nc"]

    in_maps = []
    for c in range(N_CORES):
        sl = slice(c * MPC, (c + 1) * MPC)
        in_maps.append(
            {
                "q_aug": np.ascontiguousarray(q_aug[sl]),
                "k_t": np.ascontiguousarray(kt[sl]),
                "k_nrm": np.ascontiguousarray(k_nrm[sl]),
                "v_r": np.ascontiguousarray(v_r[sl]),
                "w_aug": np.ascontiguousarray(w_aug[sl]),
            }
        )
    res = run_bass_kernel_spmd(
        nc, in_maps, list(range(N_CORES)), trace=trace
    )
    outs = np.stack([np.asarray(res.results[c]["out"]) for c in range(N_CORES)])
    full = (
        outs.reshape(NBH, ST, N_ST, E)
        .transpose(0, 2, 1, 3)
        .reshape(B, H, T, E)
        .astype(np.float32)
    )
    return full, res


def kernel(**inputs):
    full, _ = _run(inputs)
    return full


if __name__ == "__main__":
    rng = np.random.default_rng(0)
    demo = {
        "query": rng.standard_normal((B, T, H, E), dtype=np.float32),
        "key": rng.standard_normal((B, T, H, E), dtype=np.float32),
        "value": rng.standard_normal((B, T, H, E), dtype=np.float32),
        "weights": rng.standard_normal((B, H, E, R), dtype=np.float32),
    }
    out = kernel(**demo)
    print(out.shape, out.dtype)


# revision 12
# speedup vs baseline: 1.4704x; 1.4704x over previous
"""Performer attention TRN2 kernel.

Math per (b,h):
  k_feats[s,r] = exp(k[s]@W[:,r] - 0.5*||k[s]||^2)     (1/sqrt(R) dropped: cancels)
  q_feats[n,r] = exp(q[n]@W[:,r] - 0.5*||q[n]||^2)
  kv[r,d]  = sum_s k_feats[s,r] * v[s,d];  ksum[r] = sum_s k_feats[s,r]
  out[n,d] = (q_feats[n,:]@kv[:,d]) / (q_feats[n,:]@ksum)

Sharding: 16 (b,h) pairs, 2 per core across 8 cores.

Device layouts (per core, m in {0,1} is the local bh index):
  q_aug [2, 66, T]  f32r: rows 0..63 = q^T, rows 64/65 = hi/lo split of
                          -0.5*||q||^2 (applied via contraction against
                          the ones-rows of w_aug)
  k_aug [2, 66, T]  f32r: same layout for k
  v_r   [2, 128, 64*65] bf16: v with ones column, tiled [t%128, t//128, 65]
  w_aug [2, 66, R]  f32r: W with two ones-rows appended
  out   [2, 128, 64*64] f32: out tiled [t%128, t//128, 64]
"""

import sys

for _p in ("/opt/trn_rl_repo",):
    if _p not in sys.path:
        sys.path.insert(0, _p)

import numpy as np
import ml_dtypes

B, T, H, E = 2, 8192, 8, 64
R = 256
NBH = B * H  # 16
N_CORES = 8
MPC = NBH // N_CORES  # bh pairs per core = 2
ST = 128  # rows per s/n tile
N_ST = T // ST  # 64
NCHUNK = 512  # q-side moving chunk
N_CH = T // NCHUNK  # 16
KAUG = E + 2  # 66

_NC_CACHE = {}


def _build_nc():
    import concourse.bass as bass
    import concourse.mybir as mybir
    import concourse.tile as tile

    f32 = mybir.dt.float32
    f32r = mybir.dt.float32r
    bf16 = mybir.dt.bfloat16
    Exp = mybir.ActivationFunctionType.Exp
    Mult = mybir.AluOpType.mult

    nc = bass.Bass(target_bir_lowering=False)

    qa = nc.declare_dram_parameter("q_aug", [MPC, KAUG, T], f32r, isOutput=False)
    ka = nc.declare_dram_parameter("k_aug", [MPC, KAUG, T], f32r, isOutput=False)
    vr = nc.declare_dram_parameter("v_r", [MPC, ST, N_ST * 65], bf16, isOutput=False)
    wa = nc.declare_dram_parameter("w_aug", [MPC, KAUG, R], f32r, isOutput=False)
    ou = nc.declare_dram_parameter("out", [MPC, ST, N_ST * E], f32, isOutput=True)

    with tile.TileContext(nc) as tc:
        with (
            tc.tile_pool(name="io", bufs=2) as io,
            tc.tile_pool(name="feats", bufs=3) as feats,
            tc.tile_pool(name="small", bufs=2) as small,
            tc.tile_pool(name="ps", bufs=3, space="PSUM") as ps,
            tc.tile_pool(name="ops", bufs=2, space="PSUM") as ops,
            tc.tile_pool(name="acc", bufs=1, space="PSUM") as accp,
        ):
            for m in range(MPC):
                # ---- input DMAs ----
                w_sb = io.tile([KAUG, R], f32r, name="w_sb")
                nc.sync.dma_start(out=w_sb[:], in_=wa[m])
                v_sb = io.tile([ST, N_ST * 65], bf16, name="v_sb")
                for c in range(2):
                    nc.sync.dma_start(
                        out=v_sb[:, c * 2080 : (c + 1) * 2080],
                        in_=vr[m, :, c * 2080 : (c + 1) * 2080],
                    )
                k_sb = io.tile([KAUG, T], f32r, name="k_sb", bufs=1)
                for c in range(4):
                    nc.sync.dma_start(
                        out=k_sb[:, c * 2048 : (c + 1) * 2048],
                        in_=ka[m, :, c * 2048 : (c + 1) * 2048],
                    )
                q_sb = io.tile([KAUG, T], f32r, name="q_sb")
                for c in range(4):
                    nc.sync.dma_start(
                        out=q_sb[:, c * 2048 : (c + 1) * 2048],
                        in_=qa[m, :, c * 2048 : (c + 1) * 2048],
                    )

                # ---- phase A: k_feats tiles + kv/ksum accumulation ----
                kv_acc = [
                    accp.tile([ST, 65], f32, name=f"kv_acc{h}") for h in range(2)
                ]
                for g in range(N_ST // 2):
                    ps_k = ps.tile([ST, 512], f32, name="ps_proj")
                    for j in range(2):
                        st = 2 * g + j
                        nc.tensor.matmul(
                            ps_k[:, j * R : (j + 1) * R],
                            lhsT=k_sb[:, st * ST : (st + 1) * ST],
                            rhs=w_sb[:],
                            start=True,
                            stop=True,
                        )
                    kf = feats.tile([ST, 512], bf16, name="kf")
                    nc.scalar.activation(kf[:], ps_k[:], Exp)
                    for j in range(2):
                        st = 2 * g + j
                        for h in range(2):
                            nc.tensor.matmul(
                                kv_acc[h][:],
                                lhsT=kf[:, j * R + h * 128 : j * R + (h + 1) * 128],
                                rhs=v_sb[:, st * 65 : (st + 1) * 65],
                                start=(st == 0),
                                stop=(st == N_ST - 1),
                            )
                kv_sb = [small.tile([ST, 65], bf16, name=f"kv_sb{h}") for h in range(2)]
                for h in range(2):
                    nc.vector.tensor_copy(kv_sb[h][:], kv_acc[h][:])

                # ---- phase B: q_feats chunks + out tiles ----
                out_sb = io.tile([ST, N_ST * E], f32, name="out_sb")
                for ck in range(N_CH):
                    qf = []
                    for h in range(2):
                        ps_q = ps.tile([ST, NCHUNK], f32, name="ps_proj")
                        nc.tensor.matmul(
                            ps_q[:],
                            lhsT=w_sb[:, h * 128 : (h + 1) * 128],
                            rhs=q_sb[:, ck * NCHUNK : (ck + 1) * NCHUNK],
                            start=True,
                            stop=True,
                        )
                        qf_h = feats.tile([ST, NCHUNK], bf16, name="qf")
                        nc.scalar.activation(qf_h[:], ps_q[:], Exp)
                        qf.append(qf_h)
                    ps_o = ops.tile([ST, 4 * 65], f32, name="ps_o")
                    for nt in range(4):
                        for h in range(2):
                            nc.tensor.matmul(
                                ps_o[:, nt * 65 : (nt + 1) * 65],
                                lhsT=qf[h][:, nt * ST : (nt + 1) * ST],
                                rhs=kv_sb[h][:],
                                start=(h == 0),
                                stop=(h == 1),
                            )
                    rec = small.tile([ST, 4], f32, name="rec")
                    nc.vector.reciprocal(rec[:], ps_o[:, 64::65])
                    num = ps_o[:].rearrange("p (n c) -> p n c", n=4, c=65)[:, :, 0:E]
                    ov = out_sb[:, ck * 256 : (ck + 1) * 256].rearrange(
                        "p (n c) -> p n c", n=4, c=E
                    )
                    nc.vector.tensor_tensor(
                        ov, num, rec[:].unsqueeze(2).broadcast_to([ST, 4, E]), Mult
                    )
                    if (ck + 1) % 4 == 0:
                        gq = ck // 4
                        nc.sync.dma_start(
                            out=ou[m, :, gq * 1024 : (gq + 1) * 1024],
                            in_=out_sb[:, gq * 1024 : (gq + 1) * 1024],
                        )
    # TRN2 codegen allows at most 1 sem wait per instruction; split extras
    # into InstEventSemaphore (the Bacc legalization pass, safe on Bass).
    from concourse import bacc

    bacc._bass_rust.generate_event_semaphores(nc)
    return nc


def _prep_inputs(query, key, value, weights):
    query = np.asarray(query, dtype=np.float32)
    key = np.asarray(key, dtype=np.float32)
    value = np.asarray(value, dtype=np.float32)
    weights = np.asarray(weights, dtype=np.float32)

    def aug_qk(x):
        # x [B,T,H,E] -> [NBH, 66, T] with hi/lo norm rows
        xt = np.ascontiguousarray(x.transpose(0, 2, 3, 1)).reshape(NBH, E, T)
        nrm = (-0.5 * np.sum(x.astype(np.float64) * x, axis=3)).astype(np.float32)
        nrm = np.ascontiguousarray(nrm.transpose(0, 2, 1)).reshape(NBH, T)
        # hi/lo split so reduced-precision matmul modes keep the norm exact
        hi = (nrm.view(np.uint32) & np.uint32(0xFFFFE000)).view(np.float32)
        lo = nrm - hi
        aug = np.concatenate([xt, hi[:, None, :], lo[:, None, :]], axis=1)
        return np.ascontiguousarray(aug)  # [NBH, 66, T]

    q_aug = aug_qk(query)
    k_aug = aug_qk(key)

    vt = value.transpose(0, 2, 1, 3).reshape(NBH, N_ST, ST, E)
    v_aug = np.concatenate([vt, np.ones((NBH, N_ST, ST, 1), np.float32)], axis=3)
    v_r = np.ascontiguousarray(v_aug.transpose(0, 2, 1, 3)).reshape(
        NBH, ST, N_ST * 65
    ).astype(ml_dtypes.bfloat16)

    w_aug = np.concatenate(
        [weights.reshape(NBH, E, R), np.ones((NBH, 2, R), np.float32)], axis=1
    )
    w_aug = np.ascontiguousarray(w_aug)  # [NBH, 66, R]
    return q_aug, k_aug, v_r, w_aug


def _run(inputs, trace=False):
    from concourse.bass_utils import run_bass_kernel_spmd

    q_aug, k_aug, v_r, w_aug = _prep_inputs(
        inputs["query"], inputs["key"], inputs["value"], inputs["weights"]
    )
    if "nc" not in _NC_CACHE:
        _NC_CACHE["nc"] = _build_nc()
    nc = _NC_CACHE["nc"]

    in_maps = []
    for c in range(N_CORES):
        sl = slice(c * MPC, (c + 1) * MPC)
        in_maps.append(
            {
                "q_aug": np.ascontiguousarray(q_aug[sl]),
                "k_aug": np.ascontiguousarray(k_aug[sl]),
                "v_r": np.ascontiguousarray(v_r[sl]),
                "w_aug": np.ascontiguousarray(w_aug[sl]),
            }
        )
    res = run_bass_kernel_spmd(
        nc, in_maps, list(range(N_CORES)), trace=trace
    )
    outs = np.stack([np.asarray(res.results[c]["out"]) for c in range(N_CORES)])
    full = (
        outs.reshape(NBH, ST, N_ST, E)
        .transpose(0, 2, 1, 3)
        .reshape(B, H, T, E)
        .astype(np.float32)
    )
    return full, res


def kernel(**inputs):
    full, _ = _run(inputs)
    return full


if __name__ == "__main__":
    rng = np.random.default_rng(0)
    demo = {
        "query": rng.standard_normal((B, T, H, E), dtype=np.float32),
        "key": rng.standard_normal((B, T, H, E), dtype=np.float32),
        "value": rng.standard_normal((B, T, H, E), dtype=np.float32),
        "weights": rng.standard_normal((B, H, E, R), dtype=np.float32),
    }
    out = kernel(**demo)
    print(out.shape, out.dtype)


# revision 15
# speedup vs baseline: 1.5738x; 1.0704x over previous
"""Performer attention TRN2 kernel.

Math per (b,h):
  k_feats[s,r] = exp(k[s]@W[:,r] - 0.5*||k[s]||^2)     (1/sqrt(R) dropped: cancels)
  q_feats[n,r] = exp(q[n]@W[:,r] - 0.5*||q[n]||^2)
  kv[r,d]  = sum_s k_feats[s,r] * v[s,d];  ksum[r] = sum_s k_feats[s,r]
  out[n,d] = (q_feats[n,:]@kv[:,d]) / (q_feats[n,:]@ksum)

Sharding: 16 (b,h) pairs, 2 per core across 8 cores.

Device layouts (per core, m in {0,1} is the local bh index):
  q_aug [2, 66, T]  f32r: rows 0..63 = q^T, rows 64/65 = hi/lo split of
                          -0.5*||q||^2 (applied via contraction against
                          the ones-rows of w_aug)
  k_aug [2, 66, T]  f32r: same layout for k
  v_r   [2, 128, 64*65] bf16: v with ones column, tiled [t%128, t//128, 65]
  w_aug [2, 66, R]  f32r: W with two ones-rows appended
  out   [2, 128, 64*64] f32: out tiled [t%128, t//128, 64]
"""

import sys

for _p in ("/opt/trn_rl_repo",):
    if _p not in sys.path:
        sys.path.insert(0, _p)

import numpy as np
import ml_dtypes

B, T, H, E = 2, 8192, 8, 64
R = 256
NBH = B * H  # 16
N_CORES = 8
MPC = NBH // N_CORES  # bh pairs per core = 2
ST = 128  # rows per s/n tile
N_ST = T // ST  # 64
NCHUNK = 512  # q-side moving chunk
N_CH = T // NCHUNK  # 16
KAUG = E + 2  # 66

_NC_CACHE = {}


def _build_nc():
    import concourse.bass as bass
    import concourse.mybir as mybir
    import concourse.tile as tile

    f32 = mybir.dt.float32
    f32r = mybir.dt.float32r
    bf16 = mybir.dt.bfloat16
    Exp = mybir.ActivationFunctionType.Exp
    Mult = mybir.AluOpType.mult

    nc = bass.Bass(target_bir_lowering=False)

    qa = nc.declare_dram_parameter("q_aug", [MPC, KAUG, T], f32r, isOutput=False)
    ka = nc.declare_dram_parameter("k_aug", [MPC, KAUG, T], f32r, isOutput=False)
    vr = nc.declare_dram_parameter("v_r", [MPC, ST, N_ST * 65], bf16, isOutput=False)
    wa = nc.declare_dram_parameter("w_aug", [MPC, KAUG, R], f32r, isOutput=False)
    ou = nc.declare_dram_parameter("out", [MPC, ST, N_ST * E], f32, isOutput=True)

    NG = N_ST // 2  # 32 paired s-tile groups
    STAGE = 2  # kv lags proj/exp by this many groups so PE never waits on ACT

    with tile.TileContext(nc) as tc:
        with (
            tc.tile_pool(name="io", bufs=2) as io,
            tc.tile_pool(name="feats", bufs=4) as feats,
            tc.tile_pool(name="small", bufs=2) as small,
            tc.tile_pool(name="ps", bufs=3, space="PSUM") as ps,
            tc.tile_pool(name="ops", bufs=2, space="PSUM") as ops,
            tc.tile_pool(name="acc", bufs=1, space="PSUM") as accp,
        ):
            # ---- input DMAs for all m upfront (startup-critical first) ----
            sbufs = []
            for m in range(MPC):
                w_sb = io.tile([KAUG, R], f32r, name="w_sb")
                nc.sync.dma_start(out=w_sb[:], in_=wa[m])
                k_sb = io.tile([KAUG, T], f32r, name="k_sb", bufs=1)
                nc.sync.dma_start(out=k_sb[:, 0:1024], in_=ka[m, :, 0:1024])
                v_sb = io.tile([ST, N_ST * 65], bf16, name="v_sb")
                for c in range(4):
                    nc.sync.dma_start(
                        out=v_sb[:, c * 1040 : (c + 1) * 1040],
                        in_=vr[m, :, c * 1040 : (c + 1) * 1040],
                    )
                for c in range(1, 8):
                    nc.sync.dma_start(
                        out=k_sb[:, c * 1024 : (c + 1) * 1024],
                        in_=ka[m, :, c * 1024 : (c + 1) * 1024],
                    )
                q_sb = io.tile([KAUG, T], f32r, name="q_sb")
                for c in range(8):
                    nc.sync.dma_start(
                        out=q_sb[:, c * 1024 : (c + 1) * 1024],
                        in_=qa[m, :, c * 1024 : (c + 1) * 1024],
                    )
                sbufs.append((w_sb, k_sb, v_sb, q_sb))

            for m in range(MPC):
                w_sb, k_sb, v_sb, q_sb = sbufs[m]

                # ---- phase A: k_feats tiles + kv/ksum accumulation ----
                kv_acc = [
                    accp.tile([ST, 65], f32, name=f"kv_acc{h}") for h in range(2)
                ]
                kf_live = {}
                for g in range(NG + STAGE):
                    if g < NG:
                        ps_k = ps.tile([ST, 512], f32, name="ps_proj")
                        for j in range(2):
                            st = 2 * g + j
                            nc.tensor.matmul(
                                ps_k[:, j * R : (j + 1) * R],
                                lhsT=k_sb[:, st * ST : (st + 1) * ST],
                                rhs=w_sb[:],
                                start=True,
                                stop=True,
                            )
                        kf = feats.tile([ST, 512], bf16, name="kf")
                        nc.scalar.activation(kf[:], ps_k[:], Exp)
                        kf_live[g] = kf
                    if g >= STAGE:
                        kf = kf_live.pop(g - STAGE)
                        for j in range(2):
                            st = 2 * (g - STAGE) + j
                            for h in range(2):
                                nc.tensor.matmul(
                                    kv_acc[h][:],
                                    lhsT=kf[
                                        :, j * R + h * 128 : j * R + (h + 1) * 128
                                    ],
                                    rhs=v_sb[:, st * 65 : (st + 1) * 65],
                                    start=(st == 0),
                                    stop=(st == N_ST - 1),
                                )
                kv_sb = [small.tile([ST, 65], bf16, name=f"kv_sb{h}") for h in range(2)]
                for h in range(2):
                    nc.vector.tensor_copy(kv_sb[h][:], kv_acc[h][:])

                # ---- phase B: q_feats chunks + out tiles (out lags proj by 1) ----
                out_sb = io.tile([ST, N_ST * E], f32, name="out_sb")
                qf_live = {}
                for ck in range(N_CH + 1):
                    if ck < N_CH:
                        qf = []
                        for h in range(2):
                            ps_q = ps.tile([ST, NCHUNK], f32, name="ps_proj")
                            nc.tensor.matmul(
                                ps_q[:],
                                lhsT=w_sb[:, h * 128 : (h + 1) * 128],
                                rhs=q_sb[:, ck * NCHUNK : (ck + 1) * NCHUNK],
                                start=True,
                                stop=True,
                            )
                            qf_h = feats.tile([ST, NCHUNK], bf16, name=f"qf{h}")
                            nc.scalar.activation(qf_h[:], ps_q[:], Exp)
                            qf.append(qf_h)
                        qf_live[ck] = qf
                    if ck >= 1:
                        cc = ck - 1
                        qf = qf_live.pop(cc)
                        ps_o = ops.tile([ST, 4 * 65], f32, name="ps_o")
                        for nt in range(4):
                            for h in range(2):
                                nc.tensor.matmul(
                                    ps_o[:, nt * 65 : (nt + 1) * 65],
                                    lhsT=qf[h][:, nt * ST : (nt + 1) * ST],
                                    rhs=kv_sb[h][:],
                                    start=(h == 0),
                                    stop=(h == 1),
                                )
                        rec = small.tile([ST, 4], f32, name="rec")
                        nc.vector.reciprocal(rec[:], ps_o[:, 64::65])
                        num = ps_o[:].rearrange("p (n c) -> p n c", n=4, c=65)[
                            :, :, 0:E
                        ]
                        ov = out_sb[:, cc * 256 : (cc + 1) * 256].rearrange(
                            "p (n c) -> p n c", n=4, c=E
                        )
                        nc.vector.tensor_tensor(
                            ov, num, rec[:].unsqueeze(2).broadcast_to([ST, 4, E]), Mult
                        )
                        if (cc + 1) % 4 == 0:
                            gq = cc // 4
                            nc.sync.dma_start(
                                out=ou[m, :, gq * 1024 : (gq + 1) * 1024],
                                in_=out_sb[:, gq * 1024 : (gq + 1) * 1024],
                            )
    # TRN2 codegen allows at most 1 sem wait per instruction; split extras
    # into InstEventSemaphore (the Bacc legalization pass, safe on Bass).
    from concourse import bacc

    bacc._bass_rust.generate_event_semaphores(nc)
    return nc


def _prep_inputs(query, key, value, weights):
    query = np.asarray(query, dtype=np.float32)
    key = np.asarray(key, dtype=np.float32)
    value = np.asarray(value, dtype=np.float32)
    weights = np.asarray(weights, dtype=np.float32)

    def aug_qk(x):
        # x [B,T,H,E] -> [NBH, 66, T] with hi/lo norm rows
        xt = np.ascontiguousarray(x.transpose(0, 2, 3, 1)).reshape(NBH, E, T)
        nrm = (-0.5 * np.sum(x.astype(np.float64) * x, axis=3)).astype(np.float32)
        nrm = np.ascontiguousarray(nrm.transpose(0, 2, 1)).reshape(NBH, T)
        # hi/lo split so reduced-precision matmul modes keep the norm exact
        hi = (nrm.view(np.uint32) & np.uint32(0xFFFFE000)).view(np.float32)
        lo = nrm - hi
        aug = np.concatenate([xt, hi[:, None, :], lo[:, None, :]], axis=1)
        return np.ascontiguousarray(aug)  # [NBH, 66, T]

    q_aug = aug_qk(query)
    k_aug = aug_qk(key)

    vt = value.transpose(0, 2, 1, 3).reshape(NBH, N_ST, ST, E)
    v_aug = np.concatenate([vt, np.ones((NBH, N_ST, ST, 1), np.float32)], axis=3)
    v_r = np.ascontiguousarray(v_aug.transpose(0, 2, 1, 3)).reshape(
        NBH, ST, N_ST * 65
    ).astype(ml_dtypes.bfloat16)

    w_aug = np.concatenate(
        [weights.reshape(NBH, E, R), np.ones((NBH, 2, R), np.float32)], axis=1
    )
    w_aug = np.ascontiguousarray(w_aug)  # [NBH, 66, R]
    return q_aug, k_aug, v_r, w_aug


def _run(inputs, trace=False):
    from concourse.bass_utils import run_bass_kernel_spmd

    q_aug, k_aug, v_r, w_aug = _prep_inputs(
        inputs["query"], inputs["key"], inputs["value"], inputs["weights"]
    )
    if "nc" not in _NC_CACHE:
        _NC_CACHE["nc"] = _build_nc()
    nc = _NC_CACHE["nc"]

    in_maps = []
    for c in range(N_CORES):
        sl = slice(c * MPC, (c + 1) * MPC)
        in_maps.append(
            {
                "q_aug": np.ascontiguousarray(q_aug[sl]),
                "k_aug": np.ascontiguousarray(k_aug[sl]),
                "v_r": np.ascontiguousarray(v_r[sl]),
                "w_aug": np.ascontiguousarray(w_aug[sl]),
            }
        )
    res = run_bass_kernel_spmd(
        nc, in_maps, list(range(N_CORES)), trace=trace
    )
    outs = np.stack([np.asarray(res.results[c]["out"]) for c in range(N_CORES)])
    full = (
        outs.reshape(NBH, ST, N_ST, E)
        .transpose(0, 2, 1, 3)
        .reshape(B, H, T, E)
        .astype(np.float32)
    )
    return full, res


def kernel(**inputs):
    full, _ = _run(inputs)
    return full


if __name__ == "__main__":
    rng = np.random.default_rng(0)
    demo = {
        "query": rng.standard_normal((B, T, H, E), dtype=np.float32),
        "key": rng.standard_normal((B, T, H, E), dtype=np.float32),
        "value": rng.standard_normal((B, T, H, E), dtype=np.float32),
        "weights": rng.standard_normal((B, H, E, R), dtype=np.float32),
    }
    out = kernel(**demo)
    print(out.shape, out.dtype)


# revision 19
# speedup vs baseline: 1.6785x; 1.0665x over previous
"""Performer attention TRN2 kernel.

Math per (b,h):
  k_feats[s,r] = exp(k[s]@W[:,r] - 0.5*||k[s]||^2)     (1/sqrt(R) dropped: cancels)
  q_feats[n,r] = exp(q[n]@W[:,r] - 0.5*||q[n]||^2)
  kv[r,d]  = sum_s k_feats[s,r] * v[s,d];  ksum[r] = sum_s k_feats[s,r]
  out[n,d] = (q_feats[n,:]@kv[:,d]) / (q_feats[n,:]@ksum)

Sharding: 16 (b,h) pairs, 2 per core across 8 cores.

Device layouts (per core, m in {0,1} is the local bh index):
  q_aug [2, 66, T]  f32r: rows 0..63 = q^T, rows 64/65 = hi/lo split of
                          -0.5*||q||^2 (applied via contraction against
                          the ones-rows of w_aug)
  k_aug [2, 66, T]  f32r: same layout for k
  v_r   [2, 128, 64*65] bf16: v with ones column, tiled [t%128, t//128, 65]
  w_aug [2, 66, R]  f32r: W with two ones-rows appended
  out   [2, 128, 64*64] f32: out tiled [t%128, t//128, 64]
"""

import sys

for _p in ("/opt/trn_rl_repo",):
    if _p not in sys.path:
        sys.path.insert(0, _p)

import numpy as np
import ml_dtypes

B, T, H, E = 2, 8192, 8, 64
R = 256
NBH = B * H  # 16
N_CORES = 8
MPC = NBH // N_CORES  # bh pairs per core = 2
ST = 128  # rows per s/n tile
N_ST = T // ST  # 64
NCHUNK = 512  # q-side moving chunk
N_CH = T // NCHUNK  # 16
KAUG = E + 2  # 66

_NC_CACHE = {}


def _build_nc():
    import concourse.bass as bass
    import concourse.mybir as mybir
    import concourse.tile as tile

    f32 = mybir.dt.float32
    f32r = mybir.dt.float32r
    bf16 = mybir.dt.bfloat16
    Exp = mybir.ActivationFunctionType.Exp
    Mult = mybir.AluOpType.mult

    nc = bass.Bass(target_bir_lowering=False)

    qa = nc.declare_dram_parameter("q_aug", [MPC, KAUG, T], f32r, isOutput=False)
    ka = nc.declare_dram_parameter("k_aug", [MPC, KAUG, T], f32r, isOutput=False)
    vr = nc.declare_dram_parameter("v_r", [MPC, ST, N_ST * 65], bf16, isOutput=False)
    wa = nc.declare_dram_parameter("w_aug", [MPC, KAUG, R], f32r, isOutput=False)
    ou = nc.declare_dram_parameter("out", [MPC, ST, N_ST * E], f32, isOutput=True)

    NG = N_ST // 4  # 16 quad s-tile groups (1024-wide exp per group)
    STAGE = 2  # kv lags proj/exp by this many groups so PE never waits on ACT

    with tile.TileContext(nc) as tc:
        with (
            tc.tile_pool(name="io", bufs=2) as io,
            tc.tile_pool(name="feats", bufs=4) as feats,
            tc.tile_pool(name="small", bufs=2) as small,
            tc.tile_pool(name="ps", bufs=2, space="PSUM") as ps,
            tc.tile_pool(name="ops", bufs=2, space="PSUM") as ops,
            tc.tile_pool(name="acc", bufs=1, space="PSUM") as accp,
        ):
            # ---- input DMAs for all m upfront (startup-critical first) ----
            sbufs = []
            for m in range(MPC):
                w_sb = io.tile([KAUG, R], f32r, name="w_sb")
                nc.sync.dma_start(out=w_sb[:], in_=wa[m])
                k_sb = io.tile([KAUG, T], f32r, name="k_sb", bufs=1)
                v_sb = io.tile([ST, N_ST * 65], bf16, name="v_sb")
                # k in small-first pieces, v interleaved so PE starts early
                kcuts = [0, 512] + [512 + 1024 * i for i in range(1, 8)] + [T]
                vi = 0
                for c in range(len(kcuts) - 1):
                    nc.sync.dma_start(
                        out=k_sb[:, kcuts[c] : kcuts[c + 1]],
                        in_=ka[m, :, kcuts[c] : kcuts[c + 1]],
                    )
                    if c % 2 == 1 and vi < 4:
                        nc.sync.dma_start(
                            out=v_sb[:, vi * 1040 : (vi + 1) * 1040],
                            in_=vr[m, :, vi * 1040 : (vi + 1) * 1040],
                        )
                        vi += 1
                q_sb = io.tile([KAUG, T], f32r, name="q_sb")
                for c in range(8):
                    nc.sync.dma_start(
                        out=q_sb[:, c * 1024 : (c + 1) * 1024],
                        in_=qa[m, :, c * 1024 : (c + 1) * 1024],
                    )
                sbufs.append((w_sb, k_sb, v_sb, q_sb))

            for m in range(MPC):
                w_sb, k_sb, v_sb, q_sb = sbufs[m]

                # ---- phase A: k_feats tiles + kv/ksum accumulation ----
                kv_acc = [
                    accp.tile([ST, 65], f32, name=f"kv_acc{h}") for h in range(2)
                ]
                kf_live = {}
                for g in range(NG + STAGE):
                    if g < NG:
                        ps_k = ps.tile([ST, 1024], f32, name="ps_proj")
                        for j in range(4):
                            st = 4 * g + j
                            nc.tensor.matmul(
                                ps_k[:, j * R : (j + 1) * R],
                                lhsT=k_sb[:, st * ST : (st + 1) * ST],
                                rhs=w_sb[:],
                                start=True,
                                stop=True,
                            )
                        kf = feats.tile([ST, 1024], bf16, name="kf")
                        nc.scalar.activation(kf[:], ps_k[:], Exp)
                        kf_live[g] = kf
                    if g >= STAGE:
                        kf = kf_live.pop(g - STAGE)
                        for j in range(4):
                            st = 4 * (g - STAGE) + j
                            for h in range(2):
                                nc.tensor.matmul(
                                    kv_acc[h][:],
                                    lhsT=kf[
                                        :, j * R + h * 128 : j * R + (h + 1) * 128
                                    ],
                                    rhs=v_sb[:, st * 65 : (st + 1) * 65],
                                    start=(st == 0),
                                    stop=(st == N_ST - 1),
                                )
                kv_sb = [small.tile([ST, 65], bf16, name=f"kv_sb{h}") for h in range(2)]
                for h in range(2):
                    nc.vector.tensor_copy(kv_sb[h][:], kv_acc[h][:])

                # ---- phase B: q_feats chunks + out tiles (out lags proj by 1) ----
                out_sb = io.tile([ST, N_ST * E], f32, name="out_sb")
                qf_live = {}
                for ck in range(N_CH + 1):
                    if ck < N_CH:
                        ps_q = ps.tile([ST, 1024], f32, name="ps_proj")
                        for h in range(2):
                            nc.tensor.matmul(
                                ps_q[:, h * NCHUNK : (h + 1) * NCHUNK],
                                lhsT=w_sb[:, h * 128 : (h + 1) * 128],
                                rhs=q_sb[:, ck * NCHUNK : (ck + 1) * NCHUNK],
                                start=True,
                                stop=True,
                            )
                        qf = feats.tile([ST, 1024], bf16, name="qf")
                        nc.scalar.activation(qf[:], ps_q[:], Exp)
                        qf_live[ck] = qf
                    if ck >= 1:
                        cc = ck - 1
                        qf = qf_live.pop(cc)
                        ps_o = ops.tile([ST, 4 * 65], f32, name="ps_o")
                        for nt in range(4):
                            for h in range(2):
                                nc.tensor.matmul(
                                    ps_o[:, nt * 65 : (nt + 1) * 65],
                                    lhsT=qf[
                                        :,
                                        h * NCHUNK + nt * ST : h * NCHUNK
                                        + (nt + 1) * ST,
                                    ],
                                    rhs=kv_sb[h][:],
                                    start=(h == 0),
                                    stop=(h == 1),
                                )
                        rec = small.tile([ST, 4], f32, name="rec")
                        nc.vector.reciprocal(rec[:], ps_o[:, 64::65])
                        num = ps_o[:].rearrange("p (n c) -> p n c", n=4, c=65)[
                            :, :, 0:E
                        ]
                        ov = out_sb[:, cc * 256 : (cc + 1) * 256].rearrange(
                            "p (n c) -> p n c", n=4, c=E
                        )
                        nc.vector.tensor_tensor(
                            ov, num, rec[:].unsqueeze(2).broadcast_to([ST, 4, E]), Mult
                        )
                        if (cc + 1) % 2 == 0:
                            gq = cc // 2
                            nc.sync.dma_start(
                                out=ou[m, :, gq * 512 : (gq + 1) * 512],
                                in_=out_sb[:, gq * 512 : (gq + 1) * 512],
                            )
    # TRN2 codegen allows at most 1 sem wait per instruction; split extras
    # into InstEventSemaphore (the Bacc legalization pass, safe on Bass).
    from concourse import bacc

    bacc._bass_rust.generate_event_semaphores(nc)
    return nc


def _prep_inputs(query, key, value, weights):
    query = np.asarray(query, dtype=np.float32)
    key = np.asarray(key, dtype=np.float32)
    value = np.asarray(value, dtype=np.float32)
    weights = np.asarray(weights, dtype=np.float32)

    def aug_qk(x):
        # x [B,T,H,E] -> [NBH, 66, T] with hi/lo norm rows
        xt = np.ascontiguousarray(x.transpose(0, 2, 3, 1)).reshape(NBH, E, T)
        nrm = (-0.5 * np.sum(x.astype(np.float64) * x, axis=3)).astype(np.float32)
        nrm = np.ascontiguousarray(nrm.transpose(0, 2, 1)).reshape(NBH, T)
        # hi/lo split so reduced-precision matmul modes keep the norm exact
        hi = (nrm.view(np.uint32) & np.uint32(0xFFFFE000)).view(np.float32)
        lo = nrm - hi
        aug = np.concatenate([xt, hi[:, None, :], lo[:, None, :]], axis=1)
        return np.ascontiguousarray(aug)  # [NBH, 66, T]

    q_aug = aug_qk(query)
    k_aug = aug_qk(key)

    vt = value.transpose(0, 2, 1, 3).reshape(NBH, N_ST, ST, E)
    v_aug = np.concatenate([vt, np.ones((NBH, N_ST, ST, 1), np.float32)], axis=3)
    v_r = np.ascontiguousarray(v_aug.transpose(0, 2, 1, 3)).reshape(
        NBH, ST, N_ST * 65
    ).astype(ml_dtypes.bfloat16)

    w_aug = np.concatenate(
        [weights.reshape(NBH, E, R), np.ones((NBH, 2, R), np.float32)], axis=1
    )
    w_aug = np.ascontiguousarray(w_aug)  # [NBH, 66, R]
    return q_aug, k_aug, v_r, w_aug


def _run(inputs, trace=False):
    from concourse.bass_utils import run_bass_kernel_spmd

    q_aug, k_aug, v_r, w_aug = _prep_inputs(
        inputs["query"], inputs["key"], inputs["value"], inputs["weights"]
    )
    if "nc" not in _NC_CACHE:
        _NC_CACHE["nc"] = _build_nc()
    nc = _NC_CACHE["nc"]

    in_maps = []
    for c in range(N_CORES):
        sl = slice(c * MPC, (c + 1) * MPC)
        in_maps.append(
            {
                "q_aug": np.ascontiguousarray(q_aug[sl]),
                "k_aug": np.ascontiguousarray(k_aug[sl]),
                "v_r": np.ascontiguousarray(v_r[sl]),
                "w_aug": np.ascontiguousarray(w_aug[sl]),
            }
        )
    res = run_bass_kernel_spmd(
        nc, in_maps, list(range(N_CORES)), trace=trace
    )
    outs = np.stack([np.asarray(res.results[c]["out"]) for c in range(N_CORES)])
    full = (
        outs.reshape(NBH, ST, N_ST, E)
        .transpose(0, 2, 1, 3)
        .reshape(B, H, T, E)
        .astype(np.float32)
    )
    return full, res


def kernel(**inputs):
    full, _ = _run(inputs)
    return full


if __name__ == "__main__":
    rng = np.random.default_rng(0)
    demo = {
        "query": rng.standard_normal((B, T, H, E), dtype=np.float32),
        "key": rng.standard_normal((B, T, H, E), dtype=np.float32),
        "value": rng.standard_normal((B, T, H, E), dtype=np.float32),
        "weights": rng.standard_normal((B, H, E, R), dtype=np.float32),
    }
    out = kernel(**demo)
    print(out.shape, out.dtype)


# revision 23
# speedup vs baseline: 1.8655x; 1.1114x over previous
"""Performer attention TRN2 kernel.

Math per (b,h):
  k_feats[s,r] = exp(k[s]@W[:,r] - 0.5*||k[s]||^2)     (1/sqrt(R) dropped: cancels)
  q_feats[n,r] = exp(q[n]@W[:,r] - 0.5*||q[n]||^2)
  kv[r,d]  = sum_s k_feats[s,r] * v[s,d];  ksum[r] = sum_s k_feats[s,r]
  out[n,d] = (q_feats[n,:]@kv[:,d]) / (q_feats[n,:]@ksum)

Sharding: 16 (b,h) pairs, 2 per core across 8 cores.

Device layouts (per core, m in {0,1} is the local bh index):
  q_aug [2, 66, T]  f32r: rows 0..63 = q^T, rows 64/65 = hi/lo split of
                          -0.5*||q||^2 (applied via contraction against
                          the ones-rows of w_aug)
  k_aug [2, 66, T]  f32r: same layout for k
  v_r   [2, 128, 64*65] bf16: v with ones column, tiled [t%128, t//128, 65]
  w_aug [2, 66, R]  f32r: W with two ones-rows appended
  out   [2, 128, 64*64] f32: out tiled [t%128, t//128, 64]
"""

import sys

for _p in ("/opt/trn_rl_repo",):
    if _p not in sys.path:
        sys.path.insert(0, _p)

import numpy as np
import ml_dtypes

B, T, H, E = 2, 8192, 8, 64
R = 256
NBH = B * H  # 16
N_CORES = 8
MPC = NBH // N_CORES  # bh pairs per core = 2
ST = 128  # rows per s/n tile
N_ST = T // ST  # 64
NCHUNK = 512  # q-side moving chunk
N_CH = T // NCHUNK  # 16
KAUG = E + 2  # 66

_NC_CACHE = {}


def _build_nc():
    import concourse.bass as bass
    import concourse.mybir as mybir
    import concourse.tile as tile

    f32 = mybir.dt.float32
    f16 = mybir.dt.float16
    bf16 = mybir.dt.bfloat16
    Exp = mybir.ActivationFunctionType.Exp
    Mult = mybir.AluOpType.mult

    nc = bass.Bass(target_bir_lowering=False)

    qa = nc.declare_dram_parameter("q_aug", [MPC, KAUG, T], f16, isOutput=False)
    ka = nc.declare_dram_parameter("k_aug", [MPC, KAUG, T], f16, isOutput=False)
    vr = nc.declare_dram_parameter("v_r", [MPC, ST, N_ST * 65], bf16, isOutput=False)
    wa = nc.declare_dram_parameter("w_aug", [MPC, KAUG, R], f16, isOutput=False)
    ou = nc.declare_dram_parameter("out", [MPC, ST, N_ST * E], f32, isOutput=True)

    NG = N_ST // 4  # 16 quad s-tile groups (1024-wide exp per group)
    STAGE = 2  # kv lags proj/exp by this many groups so PE never waits on ACT

    with tile.TileContext(nc) as tc:
        with (
            tc.tile_pool(name="io", bufs=2) as io,
            tc.tile_pool(name="feats", bufs=4) as feats,
            tc.tile_pool(name="small", bufs=2) as small,
            tc.tile_pool(name="ps", bufs=2, space="PSUM") as ps,
            tc.tile_pool(name="ops", bufs=2, space="PSUM") as ops,
            tc.tile_pool(name="acc", bufs=1, space="PSUM") as accp,
        ):
            # ---- input DMAs for all m upfront (startup-critical first) ----
            sbufs = []
            for m in range(MPC):
                w_sb = io.tile([KAUG, R], f16, name="w_sb")
                nc.sync.dma_start(out=w_sb[:], in_=wa[m])
                k_sb = io.tile([KAUG, T], f16, name="k_sb", bufs=1)
                v_sb = io.tile([ST, N_ST * 65], bf16, name="v_sb")
                # k in small-first pieces, v interleaved so PE starts early
                kcuts = [0, 512] + [512 + 1024 * i for i in range(1, 8)] + [T]
                vi = 0
                for c in range(len(kcuts) - 1):
                    nc.sync.dma_start(
                        out=k_sb[:, kcuts[c] : kcuts[c + 1]],
                        in_=ka[m, :, kcuts[c] : kcuts[c + 1]],
                    )
                    if c % 2 == 1 and vi < 4:
                        nc.sync.dma_start(
                            out=v_sb[:, vi * 1040 : (vi + 1) * 1040],
                            in_=vr[m, :, vi * 1040 : (vi + 1) * 1040],
                        )
                        vi += 1
                q_sb = io.tile([KAUG, T], f16, name="q_sb")
                for c in range(4):
                    nc.sync.dma_start(
                        out=q_sb[:, c * 2048 : (c + 1) * 2048],
                        in_=qa[m, :, c * 2048 : (c + 1) * 2048],
                    )
                sbufs.append((w_sb, k_sb, v_sb, q_sb))

            for m in range(MPC):
                w_sb, k_sb, v_sb, q_sb = sbufs[m]

                # ---- phase A: k_feats tiles + kv/ksum accumulation ----
                kv_acc = [
                    accp.tile([ST, 65], f32, name=f"kv_acc{h}") for h in range(2)
                ]
                kf_live = {}
                for g in range(NG + STAGE):
                    if g < NG:
                        ps_k = ps.tile([ST, 1024], f32, name="ps_proj")
                        for j in range(4):
                            st = 4 * g + j
                            nc.tensor.matmul(
                                ps_k[:, j * R : (j + 1) * R],
                                lhsT=k_sb[:, st * ST : (st + 1) * ST],
                                rhs=w_sb[:],
                                start=True,
                                stop=True,
                            )
                        kf = feats.tile([ST, 1024], bf16, name="kf")
                        nc.scalar.activation(kf[:], ps_k[:], Exp)
                        kf_live[g] = kf
                    if g >= STAGE:
                        kf = kf_live.pop(g - STAGE)
                        for j in range(4):
                            st = 4 * (g - STAGE) + j
                            for h in range(2):
                                nc.tensor.matmul(
                                    kv_acc[h][:],
                                    lhsT=kf[
                                        :, j * R + h * 128 : j * R + (h + 1) * 128
                                    ],
                                    rhs=v_sb[:, st * 65 : (st + 1) * 65],
                                    start=(st == 0),
                                    stop=(st == N_ST - 1),
                                )
                kv_sb = [small.tile([ST, 65], bf16, name=f"kv_sb{h}") for h in range(2)]
                for h in range(2):
                    nc.vector.tensor_copy(kv_sb[h][:], kv_acc[h][:])

                # ---- phase B: q_feats chunks + out tiles (out lags proj by 1) ----
                out_sb = io.tile([ST, N_ST * E], f32, name="out_sb")
                qf_live = {}
                for ck in range(N_CH + 1):
                    if ck < N_CH:
                        ps_q = ps.tile([ST, 1024], f32, name="ps_proj")
                        for h in range(2):
                            nc.tensor.matmul(
                                ps_q[:, h * NCHUNK : (h + 1) * NCHUNK],
                                lhsT=w_sb[:, h * 128 : (h + 1) * 128],
                                rhs=q_sb[:, ck * NCHUNK : (ck + 1) * NCHUNK],
                                start=True,
                                stop=True,
                            )
                        qf = feats.tile([ST, 1024], bf16, name="qf")
                        nc.scalar.activation(qf[:], ps_q[:], Exp)
                        qf_live[ck] = qf
                    if ck >= 1:
                        cc = ck - 1
                        qf = qf_live.pop(cc)
                        ps_o = ops.tile([ST, 4 * 65], f32, name="ps_o")
                        for nt in range(4):
                            for h in range(2):
                                nc.tensor.matmul(
                                    ps_o[:, nt * 65 : (nt + 1) * 65],
                                    lhsT=qf[
                                        :,
                                        h * NCHUNK + nt * ST : h * NCHUNK
                                        + (nt + 1) * ST,
                                    ],
                                    rhs=kv_sb[h][:],
                                    start=(h == 0),
                                    stop=(h == 1),
                                )
                        rec = small.tile([ST, 4], f32, name="rec")
                        nc.vector.reciprocal(rec[:], ps_o[:, 64::65])
                        num = ps_o[:].rearrange("p (n c) -> p n c", n=4, c=65)[
                            :, :, 0:E
                        ]
                        ov = out_sb[:, cc * 256 : (cc + 1) * 256].rearrange(
                            "p (n c) -> p n c", n=4, c=E
                        )
                        nc.vector.tensor_tensor(
                            ov, num, rec[:].unsqueeze(2).broadcast_to([ST, 4, E]), Mult
                        )
                        if (cc + 1) % 2 == 0:
                            gq = cc // 2
                            nc.sync.dma_start(
                                out=ou[m, :, gq * 512 : (gq + 1) * 512],
                                in_=out_sb[:, gq * 512 : (gq + 1) * 512],
                            )
    # TRN2 codegen allows at most 1 sem wait per instruction; split extras
    # into InstEventSemaphore (the Bacc legalization pass, safe on Bass).
    from concourse import bacc

    bacc._bass_rust.generate_event_semaphores(nc)
    return nc


def _prep_inputs(query, key, value, weights):
    query = np.asarray(query, dtype=np.float32)
    key = np.asarray(key, dtype=np.float32)
    value = np.asarray(value, dtype=np.float32)
    weights = np.asarray(weights, dtype=np.float32)

    def aug_qk(x):
        # x [B,T,H,E] -> [NBH, 66, T] fp16 with hi/lo norm rows
        xt = np.ascontiguousarray(x.transpose(0, 2, 3, 1)).reshape(NBH, E, T)
        nrm = (-0.5 * np.sum(x.astype(np.float64) * x, axis=3)).astype(np.float32)
        nrm = np.ascontiguousarray(nrm.transpose(0, 2, 1)).reshape(NBH, T)
        # hi/lo split at fp16 granularity keeps the norm near-exact
        hi = nrm.astype(np.float16)
        lo = (nrm - hi.astype(np.float32)).astype(np.float16)
        aug = np.concatenate(
            [xt.astype(np.float16), hi[:, None, :], lo[:, None, :]], axis=1
        )
        return np.ascontiguousarray(aug)  # [NBH, 66, T] fp16

    q_aug = aug_qk(query)
    k_aug = aug_qk(key)

    vt = value.transpose(0, 2, 1, 3).reshape(NBH, N_ST, ST, E)
    v_aug = np.concatenate([vt, np.ones((NBH, N_ST, ST, 1), np.float32)], axis=3)
    v_r = np.ascontiguousarray(v_aug.transpose(0, 2, 1, 3)).reshape(
        NBH, ST, N_ST * 65
    ).astype(ml_dtypes.bfloat16)

    w_aug = np.concatenate(
        [weights.reshape(NBH, E, R), np.ones((NBH, 2, R), np.float32)], axis=1
    )
    w_aug = np.ascontiguousarray(w_aug).astype(np.float16)  # [NBH, 66, R]
    return q_aug, k_aug, v_r, w_aug


def _run(inputs, trace=False):
    from concourse.bass_utils import run_bass_kernel_spmd

    q_aug, k_aug, v_r, w_aug = _prep_inputs(
        inputs["query"], inputs["key"], inputs["value"], inputs["weights"]
    )
    if "nc" not in _NC_CACHE:
        _NC_CACHE["nc"] = _build_nc()
    nc = _NC_CACHE["nc"]

    in_maps = []
    for c in range(N_CORES):
        sl = slice(c * MPC, (c + 1) * MPC)
        in_maps.append(
            {
                "q_aug": np.ascontiguousarray(q_aug[sl]),
                "k_aug": np.ascontiguousarray(k_aug[sl]),
                "v_r": np.ascontiguousarray(v_r[sl]),
                "w_aug": np.ascontiguousarray(w_aug[sl]),
            }
        )
    res = run_bass_kernel_spmd(
        nc, in_maps, list(range(N_CORES)), trace=trace
    )
    outs = np.stack([np.asarray(res.results[c]["out"]) for c in range(N_CORES)])
    full = (
        outs.reshape(NBH, ST, N_ST, E)
        .transpose(0, 2, 1, 3)
        .reshape(B, H, T, E)
        .astype(np.float32)
    )
    return full, res


def kernel(**inputs):
    full, _ = _run(inputs)
    return full


if __name__ == "__main__":
    rng = np.random.default_rng(0)
    demo = {
        "query": rng.standard_normal((B, T, H, E), dtype=np.float32),
        "key": rng.standard_normal((B, T, H, E), dtype=np.float32),
        "value": rng.standard_normal((B, T, H, E), dtype=np.float32),
        "weights": rng.standard_normal((B, H, E, R), dtype=np.float32),
    }
    out = kernel(**demo)
    print(out.shape, out.dtype)
